# revision 29
# baseline (speedup 1.0000x reference)
"""Trainium2 Bass kernel for CSPCPCPNet-style GNN message passing.

Graph structure (from the model): B=128 independent graphs, 32 nodes each,
fully-connected edges (incl. self-loops) that never cross graphs, nodes/edges
laid out contiguously per graph.  Edge e = g*1024 + i*32 + j has src=g*32+i,
dst=g*32+j.  The output only depends on the *set* of edges (aggregations are
permutation invariant), so the kernel uses this structure directly.

Sharding: 16 graphs per NeuronCore x 8 cores, MLP weights replicated,
no collectives.  Everything on-chip is kept transposed: features on the
128 partitions, edges/nodes along the free dimension.
"""

import os
import numpy as np
from contextlib import ExitStack

H = 128
L = 4
B = 128
NPG = 32
EPG = NPG * NPG  # 1024
NFREQ = 10
NCORES = 8
BPC = B // NCORES  # 16 graphs per core
NPC = BPC * NPG  # 512 nodes per core
WAVES = BPC // 4  # waves of 4 graphs

F32R_BIG = True  # use float32r (reduced precision, ~4x faster) for big matmuls


# ----------------------------------------------------------------------------
# host-side constant / weight packing (all arrays already in SBUF layout [P, F])
# ----------------------------------------------------------------------------

def _build_consts():
    c = {}
    # ABsel [64, 1024]: rows 0-31 select src i, rows 32-63 select dst j
    absel = np.zeros((64, EPG), np.float32)
    for i in range(NPG):
        absel[i, i * NPG:(i + 1) * NPG] = 1.0
        absel[32 + i, i::NPG] = 1.0
    c["abselc"] = absel
    # Rf60 [3, 60]: Rf60[d, d*10+k] = Rf60[d, 30+d*10+k] = k  (frequency / 2pi).
    # The kernel computes t = k*(x_j - x_i) + 16(+0.25 for cos) > 0, reduces
    # z = t mod 1, and evaluates sin(2*pi*z - pi) = -sin(2*pi*k*dx (+ pi/2));
    # the leading minus is folded into the (negated) W1d weights.
    rf = np.zeros((3, 60), np.float32)
    for d in range(3):
        for k in range(NFREQ):
            rf[d, d * NFREQ + k] = float(k)
            rf[d, 30 + d * NFREQ + k] = float(k)
    c["rf60"] = rf
    offv = np.full((60, 1), 16.0, np.float32)
    offv[30:] = 16.25
    c["offv"] = offv
    return c


def _pack_weights(edge_w1, edge_b1, edge_w2, edge_b2,
                  node_w1, node_b1, node_w2, node_b2, node_emb, out_w):
    w = {}
    w1ab = np.zeros((H, L * 256), np.float32)
    w1dz = np.zeros((64, L * H), np.float32)
    w1cb = np.zeros((10, L * H), np.float32)
    w2p = np.zeros((H, L * H), np.float32)
    nw1 = np.zeros((H, L * 256), np.float32)
    nw2 = np.zeros((H, L * H), np.float32)
    for l in range(L):
        w1ab[:, 256 * l:256 * l + 128] = edge_w1[l][:128, :]
        w1ab[:, 256 * l + 128:256 * l + 256] = edge_w1[l][128:256, :]
        w1dz[:60, H * l:H * (l + 1)] = -edge_w1[l][265:325, :]
        w1cb[:9, H * l:H * (l + 1)] = edge_w1[l][256:265, :]
        w1cb[9, H * l:H * (l + 1)] = edge_b1[l]
        w2p[:, H * l:H * (l + 1)] = edge_w2[l]
        nw1[:, 256 * l:256 * l + 128] = node_w1[l][:128, :]
        nw1[:, 256 * l + 128:256 * l + 256] = node_w1[l][128:, :] / 32.0
        nw2[:, H * l:H * (l + 1)] = node_w2[l]
    w["w1ab"] = w1ab
    w["w1dz"] = w1dz
    w["w1cb"] = w1cb
    w["w2p"] = w2p
    w["nw1"] = nw1
    w["nw2"] = nw2
    w["b2t"] = np.ascontiguousarray(edge_b2.T)    # [128, 4]
    w["nb1t"] = np.ascontiguousarray(node_b1.T)   # [128, 4]
    w["nb2t"] = np.ascontiguousarray(node_b2.T)   # [128, 4]
    w["nemb"] = np.ascontiguousarray(node_emb)    # [100, 128]
    w["outw"] = np.ascontiguousarray(out_w / 32.0)
    return w


def _per_core_inputs(core, atom_types, frac_coords, lattices):
    d = {}
    ns = slice(NPC * core, NPC * (core + 1))
    gs = slice(BPC * core, BPC * (core + 1))
    d["fract"] = np.ascontiguousarray(frac_coords[ns].T)  # [3, 512]
    oh = np.zeros((100, NPC), np.float32)
    at = atom_types[ns].astype(np.int64) - 1
    oh[at, np.arange(NPC)] = 1.0
    d["onehott"] = oh
    A = lattices[gs]  # [16, 3, 3]
    lra = np.zeros((10, 3 * BPC), np.float32)
    lrb = np.zeros((10, 3 * BPC), np.float32)
    lra[:9] = np.broadcast_to(A.transpose(1, 0, 2)[:, None, :, :],
                              (3, 3, BPC, 3)).reshape(9, 3 * BPC)
    lrb[:9] = np.broadcast_to(A.transpose(1, 0, 2)[None, :, :, :],
                              (3, 3, BPC, 3)).reshape(9, 3 * BPC)
    # row 9 produces the constant-one row of vall after the j-reduce
    lra[9, 0::3] = 1.0
    lrb[9, 0::3] = 1.0
    d["lra"] = lra
    d["lrb"] = lrb
    return d


_SHAPES = dict(
    fract=(3, NPC), onehott=(100, NPC), lra=(10, 3 * BPC), lrb=(10, 3 * BPC),
    abselc=(64, EPG), rf60=(3, 60), offv=(60, 1),
    w1ab=(H, L * 256), w1dz=(64, L * H), w1cb=(10, L * H), w2p=(H, L * H),
    nw1=(H, L * 256), nw2=(H, L * H),
    b2t=(H, L), nb1t=(H, L), nb2t=(H, L),
    nemb=(100, H), outw=(H, H),
)


# ----------------------------------------------------------------------------
# device kernel
# ----------------------------------------------------------------------------

def _emit(tc, nc, sbin, out_dram, ctx):
    import concourse.bass as bass
    from concourse import mybir

    f32 = mybir.dt.float32
    f32r = mybir.dt.float32r
    AF = mybir.ActivationFunctionType
    ALU = mybir.AluOpType
    AX = mybir.AxisListType

    # dtype for tensors consumed by the big matmuls: walrus requires fp32r
    # operands to be *produced* as fp32r, so the tiles carry the dtype.
    fbig = f32r if F32R_BIG else f32

    singles = ctx.enter_context(tc.tile_pool(name="singles", bufs=1))
    sigp = ctx.enter_context(tc.tile_pool(name="sigp", bufs=3))
    work = ctx.enter_context(tc.tile_pool(name="work", bufs=3))
    hpool = ctx.enter_context(tc.tile_pool(name="hpool", bufs=3))
    eps_pool = ctx.enter_context(tc.tile_pool(name="eps", bufs=2, space="PSUM"))
    sps_pool = ctx.enter_context(tc.tile_pool(name="sps", bufs=2, space="PSUM"))

    # ---- load all weights/constants into SBUF --------------------------------
    sb = {}
    for name, shape in _SHAPES.items():
        if name == "abselc":
            continue  # DMA'd straight into the per-graph disab blocks
        dt = fbig if name in ("w1dz", "w2p") else f32
        t = singles.tile(list(shape), dt, name=f"sb_{name}")
        nc.sync.dma_start(out=t, in_=sbin[name].ap())
        sb[name] = t

    # disAB: per graph [128, 1024]; rows 0-59 sin-embedding (written later),
    # rows 60-63 zero, rows 64-127 the A/B one-hot selector.
    disab = singles.tile([128, BPC * EPG], fbig, name="disab")
    for g in range(BPC):
        nc.sync.dma_start(out=disab[64:128, EPG * g:EPG * (g + 1)],
                          in_=sbin["abselc"].ap())
    # zero rows 60-63 (Sin overwrites 32-59 afterwards; 32-aligned start needed;
    # uint32 view: DVE memset may not produce float32r directly)
    nc.vector.memset(disab[32:64, :].bitcast(mybir.dt.uint32), 0)
    zero60 = singles.tile([60, 1], f32, name="zero60")
    nc.vector.memset(zero60, 0.0)

    # ---- phase 0: sinusoid embeddings (all Sin before any Silu) --------------
    # fracrT[r, n] = k_r * frac[n, d_r];  u = fracrT + off (16 / 16.25 cos);
    # t[r,(i,j)] = u[:,j] - fracrT[:,i] in (6, 27);
    # b = (t + 2^23) - 2^23 rounds t to the nearest integer (fp32 trick);
    # zneg = b - t in [-0.5, 0.5];  sin(2*pi*zneg) = -sin(2*pi*k*dx (+pi/2)),
    # and the leading minus is folded into the (negated) W1d weights.
    RC = float(2 ** 23)
    for g in range(BPC):
        fr_ps = sps_pool.tile([60, NPG], f32, tag="ab", name="fr_ps")
        nc.tensor.matmul(fr_ps, lhsT=sb["rf60"],
                         rhs=sb["fract"][:, NPG * g:NPG * (g + 1)])
        fracrt = work.tile([60, NPG], f32, tag="fracrt", name="fracrt")
        nc.vector.tensor_copy(fracrt, fr_ps)
        uoff = work.tile([60, NPG], f32, tag="uoff", name="uoff")
        nc.vector.tensor_scalar_add(uoff, fracrt, sb["offv"])
        bcast_j = bass.AP(tensor=uoff.tensor, offset=uoff.offset,
                          ap=[uoff.ap[0], [0, NPG], [1, NPG]])
        bcast_i = bass.AP(tensor=fracrt.tensor, offset=fracrt.offset,
                          ap=[fracrt.ap[0], [1, NPG], [0, NPG]])
        tt = work.tile([60, NPG, NPG], f32, tag="tt", name="tt")
        nc.gpsimd.tensor_sub(tt, bcast_j, bcast_i)
        tb = work.tile([60, EPG], f32, tag="tb", name="tb")
        nc.vector.tensor_scalar(tb.rearrange("p (i j) -> p i j", j=NPG), tt,
                                RC, RC, op0=ALU.add, op1=ALU.subtract)
        tz = work.tile([60, EPG], f32, tag="tz", name="tz")
        nc.gpsimd.tensor_sub(tz, tb,
                             tt.rearrange("p i j -> p (i j)"))
        nc.scalar.activation(out=disab[0:60, EPG * g:EPG * (g + 1)], in_=tz,
                             func=AF.Sin, bias=zero60,
                             scale=2.0 * float(np.pi))

    # ---- phase 0b: h init (embedding gather via one-hot matmul) --------------
    h4_ps = sps_pool.tile([H, NPC], f32, tag="node", name="h4_ps")
    nc.tensor.matmul(h4_ps, lhsT=sb["nemb"], rhs=sb["onehott"])
    hts = [[None] * (L + 1) for _ in range(WAVES)]
    for w in range(WAVES):
        ht0 = hpool.tile([H, 128], f32, tag=f"ht{w}", name=f"ht_{w}_0")
        nc.vector.tensor_copy(ht0, h4_ps[:, 128 * w:128 * (w + 1)])
        hts[w][0] = ht0

    # ---- phase 0c: lattice inner products -> per-(graph,layer) act biases ----
    vtmp = singles.tile([10, 3 * BPC], f32, name="vtmp")
    nc.vector.tensor_mul(vtmp, sb["lra"], sb["lrb"])
    vall = singles.tile([10, BPC], f32, name="vall")
    nc.vector.tensor_reduce(out=vall,
                            in_=vtmp.rearrange("p (b j) -> p b j", j=3),
                            axis=AX.X, op=ALU.add)
    biast = singles.tile([H, L * BPC], f32, name="biast")
    for l in range(L):
        b_ps = sps_pool.tile([H, BPC], f32, tag="ab", name="b_ps")
        nc.tensor.matmul(b_ps, lhsT=sb["w1cb"][:, H * l:H * (l + 1)], rhs=vall)
        nc.vector.tensor_copy(biast[:, BPC * l:BPC * (l + 1)], b_ps)

    # ---- phase 1: L rounds of message passing, waves of 4 graphs -------------
    for w in range(WAVES):
        for l in range(L):
            ht = hts[w][l]
            agg = work.tile([H, 128], f32, tag="agg", name="agg")
            for g4 in range(4):
                g = 4 * w + g4
                # A/B node-feature matmuls into PSUM partitions 64..127
                ab_ps = sps_pool.tile([128, H], f32, tag="ab", name="ab_ps")
                nc.tensor.matmul(ab_ps[64:96, :],
                                 lhsT=ht[:, 32 * g4:32 * g4 + 32],
                                 rhs=sb["w1ab"][:, 256 * l:256 * l + 128],
                                 tile_position=(0, 64))
                nc.tensor.matmul(ab_ps[96:128, :],
                                 lhsT=ht[:, 32 * g4:32 * g4 + 32],
                                 rhs=sb["w1ab"][:, 256 * l + 128:256 * l + 256],
                                 tile_position=(0, 96))
                lhstp = work.tile([128, H], fbig, tag="lhstp", name="lhstp")
                nc.gpsimd.tensor_copy(lhstp[0:64, :],
                                      sb["w1dz"][:, H * l:H * (l + 1)])
                nc.vector.tensor_copy(lhstp[64:128, :], ab_ps[64:128, :])
                # pre-activation edge features [128, 1024]
                pre_ps = eps_pool.tile([H, EPG], f32, tag="pre", name="pre_ps")
                for cch in range(2):
                    cs = slice(512 * cch, 512 * (cch + 1))
                    nc.tensor.matmul(pre_ps[:, cs], lhsT=lhstp,
                                     rhs=disab[:, EPG * g + 512 * cch:
                                               EPG * g + 512 * (cch + 1)])
                sig1 = sigp.tile([H, EPG], fbig, tag="sig1", name="sig1")
                nc.scalar.activation(out=sig1, in_=pre_ps, func=AF.Silu,
                                     bias=biast[:, BPC * l + g:BPC * l + g + 1],
                                     scale=1.0)
                m2_ps = eps_pool.tile([H, EPG], f32, tag="pre", name="m2_ps")
                for cch in range(2):
                    cs = slice(512 * cch, 512 * (cch + 1))
                    nc.tensor.matmul(m2_ps[:, cs],
                                     lhsT=sb["w2p"][:, H * l:H * (l + 1)],
                                     rhs=sig1[:, cs])
                sig2 = sigp.tile([H, EPG], f32, tag="sig2", name="sig2")
                nc.scalar.activation(out=sig2, in_=m2_ps, func=AF.Silu,
                                     bias=sb["b2t"][:, l:l + 1], scale=1.0)
                nc.vector.tensor_reduce(
                    out=agg[:, 32 * g4:32 * g4 + 32],
                    in_=sig2.rearrange("p (i j) -> p i j", j=NPG),
                    axis=AX.X, op=ALU.add)
            # node update, 4 graphs at once
            u1_ps = sps_pool.tile([H, 128], f32, tag="node", name="u1_ps")
            nc.tensor.matmul(u1_ps, lhsT=sb["nw1"][:, 256 * l:256 * l + 128],
                             rhs=ht, start=True, stop=False)
            nc.tensor.matmul(u1_ps, lhsT=sb["nw1"][:, 256 * l + 128:256 * l + 256],
                             rhs=agg, start=False, stop=True)
            u1 = work.tile([H, 128], f32, tag="u1", name="u1")
            nc.scalar.activation(out=u1, in_=u1_ps, func=AF.Silu,
                                 bias=sb["nb1t"][:, l:l + 1], scale=1.0)
            u2_ps = sps_pool.tile([H, 128], f32, tag="node", name="u2_ps")
            nc.tensor.matmul(u2_ps, lhsT=sb["nw2"][:, H * l:H * (l + 1)], rhs=u1)
            u2 = work.tile([H, 128], f32, tag="u2", name="u2")
            nc.scalar.activation(out=u2, in_=u2_ps, func=AF.Silu,
                                 bias=sb["nb2t"][:, l:l + 1], scale=1.0)
            htn = hpool.tile([H, 128], f32, tag=f"ht{w}", name=f"ht_{w}_{l + 1}")
            nc.gpsimd.tensor_add(htn, ht, u2)
            hts[w][l + 1] = htn

    # ---- phase 2: graph pooling + output projection --------------------------
    gt = singles.tile([H, BPC], f32, name="gt")
    for w in range(WAVES):
        nc.vector.tensor_reduce(
            out=gt[:, 4 * w:4 * (w + 1)],
            in_=hts[w][L].rearrange("p (b n) -> p b n", n=NPG),
            axis=AX.X, op=ALU.add)
    out_ps = sps_pool.tile([H, BPC], f32, tag="ab", name="out_ps")
    nc.tensor.matmul(out_ps, lhsT=sb["outw"], rhs=gt)
    outsb = singles.tile([H, BPC], f32, name="outsb")
    nc.vector.tensor_copy(outsb, out_ps)
    nc.sync.dma_start(out=out_dram.ap(), in_=outsb)


def _build():
    import concourse.bass as bass
    import concourse.bacc as bacc
    import concourse.tile as tile
    from concourse import mybir

    nc = bacc.Bacc("TRN2", target_bir_lowering=False, debug=False,
                   enable_asserts=False, num_devices=NCORES)
    fbig = mybir.dt.float32r if F32R_BIG else mybir.dt.float32
    sbin = {name: nc.dram_tensor(
                name, list(shape),
                fbig if name in ("w1dz", "w2p", "abselc") else mybir.dt.float32,
                kind="ExternalInput")
            for name, shape in _SHAPES.items()}
    out_dram = nc.dram_tensor("outt", [H, BPC], mybir.dt.float32,
                              kind="ExternalOutput")
    with tile.TileContext(nc) as tc:
        with ExitStack() as ctx:
            _emit(tc, nc, sbin, out_dram, ctx)
    nc.compile()
    from concourse.bass_interp import get_hw_module
    nc.m = get_hw_module(nc.m)
    return nc


_NC = None


def _get_nc():
    global _NC
    if _NC is None:
        _NC = _build()
    return _NC


def _make_in_maps(inputs):
    atom_types = np.asarray(inputs["atom_types"]).astype(np.int32)
    frac_coords = np.asarray(inputs["frac_coords"]).astype(np.float32)
    lattices = np.asarray(inputs["lattices"]).astype(np.float32)
    shared = {}
    shared.update(_build_consts())
    shared.update(_pack_weights(
        np.asarray(inputs["edge_w1"], np.float32),
        np.asarray(inputs["edge_b1"], np.float32),
        np.asarray(inputs["edge_w2"], np.float32),
        np.asarray(inputs["edge_b2"], np.float32),
        np.asarray(inputs["node_w1"], np.float32),
        np.asarray(inputs["node_b1"], np.float32),
        np.asarray(inputs["node_w2"], np.float32),
        np.asarray(inputs["node_b2"], np.float32),
        np.asarray(inputs["node_emb"], np.float32),
        np.asarray(inputs["out_w"], np.float32)))
    in_maps = []
    for core in range(NCORES):
        m = dict(shared)
        m.update(_per_core_inputs(core, atom_types, frac_coords, lattices))
        for k in m:
            m[k] = np.ascontiguousarray(m[k], dtype=np.float32)
        in_maps.append(m)
    return in_maps


_EXEC = None


def _get_exec():
    """Build (once) a jitted PJRT callable running the NEFF on all 8 cores."""
    global _EXEC
    if _EXEC is not None:
        return _EXEC
    import jax
    from jax.sharding import Mesh, PartitionSpec
    from jax.experimental.shard_map import shard_map
    from concourse import bass2jax, mybir

    bass2jax.install_neuronx_cc_hook()
    nc = _get_nc()
    partition_name = (nc.partition_id_tensor.name
                      if nc.partition_id_tensor else None)
    in_names, out_names, out_avals = [], [], []
    for alloc in nc.m.functions[0].allocations:
        if not isinstance(alloc, mybir.MemoryLocationSet):
            continue
        name = alloc.memorylocations[0].name
        if alloc.kind == "ExternalInput":
            if name != partition_name:
                in_names.append(name)
        elif alloc.kind == "ExternalOutput":
            out_names.append(name)
            out_avals.append(jax.core.ShapedArray(
                tuple(alloc.tensor_shape), mybir.dt.np(alloc.dtype)))
    n_params = len(in_names)
    all_in_names = list(in_names) + list(out_names)
    if partition_name is not None:
        all_in_names.append(partition_name)

    def _body(*args):
        operands = list(args)
        if partition_name is not None:
            operands.append(bass2jax.partition_id_tensor())
        outs = bass2jax._bass_exec_p.bind(
            *operands,
            out_avals=tuple(out_avals),
            in_names=tuple(all_in_names),
            out_names=tuple(out_names),
            lowering_input_output_aliases=(),
            sim_require_finite=True,
            sim_require_nnan=True,
            nc=nc,
        )
        return tuple(outs)

    devices = jax.devices()[:NCORES]
    mesh = Mesh(np.asarray(devices), ("core",))
    n_outs = len(out_names)
    in_specs = (PartitionSpec("core"),) * (n_params + n_outs)
    out_specs = (PartitionSpec("core"),) * n_outs
    fn = jax.jit(shard_map(_body, mesh=mesh, in_specs=in_specs,
                           out_specs=out_specs, check_rep=False),
                 keep_unused=True)
    _EXEC = (fn, in_names, out_names, out_avals, mesh)
    return _EXEC


def _device_args(inputs):
    import jax
    from jax.sharding import NamedSharding, PartitionSpec
    fn, in_names, out_names, out_avals, mesh = _get_exec()
    in_maps = _make_in_maps(inputs)
    concat_in = [np.concatenate([in_maps[c][name] for c in range(NCORES)],
                                axis=0) for name in in_names]
    concat_zeros = [np.zeros((NCORES * a.shape[0], *a.shape[1:]), a.dtype)
                    for a in out_avals]
    sh = NamedSharding(mesh, PartitionSpec("core"))
    return [jax.device_put(a, sh) for a in concat_in + concat_zeros]


def _gather_out(out_arrs):
    outt = np.asarray(out_arrs[0]).reshape(NCORES, H, BPC)
    out = np.zeros((B, H), np.float32)
    for core in range(NCORES):
        out[BPC * core:BPC * (core + 1), :] = outt[core].T
    return out


def _run(inputs):
    import jax
    fn = _get_exec()[0]
    args = _device_args(inputs)
    out_arrs = fn(*args)
    jax.block_until_ready(out_arrs)
    return _gather_out(out_arrs), (fn, args)


def kernel(**inputs) -> np.ndarray:
    out, _ = _run(inputs)
    return out


# revision 45
# speedup vs baseline: 1.0923x; 1.0923x over previous
"""Trainium2 Bass kernel for CSPCPCPNet-style GNN message passing.

Graph structure (from the model): B=128 independent graphs, 32 nodes each,
fully-connected edges (incl. self-loops) that never cross graphs, nodes/edges
laid out contiguously per graph.  Edge e = g*1024 + i*32 + j has src=g*32+i,
dst=g*32+j.  The output only depends on the *set* of edges (aggregations are
permutation invariant), so the kernel uses this structure directly.

Sharding: 16 graphs per NeuronCore x 8 cores, MLP weights replicated,
no collectives.  Everything on-chip is kept transposed: features on the
128 partitions, edges/nodes along the free dimension.
"""

import os
import numpy as np
from contextlib import ExitStack

H = 128
L = 4
B = 128
NPG = 32
EPG = NPG * NPG  # 1024
NFREQ = 10
NCORES = 8
BPC = B // NCORES  # 16 graphs per core
NPC = BPC * NPG  # 512 nodes per core
WAVES = BPC // 4  # waves of 4 graphs

F32R_BIG = True  # use float32r (reduced precision, ~4x faster) for big matmuls


# ----------------------------------------------------------------------------
# host-side constant / weight packing (all arrays already in SBUF layout [P, F])
# ----------------------------------------------------------------------------

def _build_consts():
    c = {}
    # ABsel [64, 1024]: rows 0-31 select src i, rows 32-63 select dst j
    absel = np.zeros((64, EPG), np.float32)
    for i in range(NPG):
        absel[i, i * NPG:(i + 1) * NPG] = 1.0
        absel[32 + i, i::NPG] = 1.0
    c["abselc"] = absel
    # Rf60 [3, 60]: Rf60[d, d*10+k] = Rf60[d, 30+d*10+k] = k  (frequency / 2pi).
    # The kernel computes t = k*(x_j - x_i) + 16(+0.25 for cos) > 0, reduces
    # z = t mod 1, and evaluates sin(2*pi*z - pi) = -sin(2*pi*k*dx (+ pi/2));
    # the leading minus is folded into the (negated) W1d weights.
    rf = np.zeros((3, 60), np.float32)
    for d in range(3):
        for k in range(NFREQ):
            rf[d, d * NFREQ + k] = float(k)
            rf[d, 30 + d * NFREQ + k] = float(k)
    c["rf60"] = rf
    offv = np.full((60, 1), 16.0, np.float32)
    offv[30:] = 16.25
    c["offv"] = offv
    c["zer32"] = np.zeros((32, EPG), np.float32)
    return c


def _pack_weights(edge_w1, edge_b1, edge_w2, edge_b2,
                  node_w1, node_b1, node_w2, node_b2, node_emb, out_w):
    w = {}
    w1ab = np.zeros((H, L * 256), np.float32)
    w1dz = np.zeros((64, L * H), np.float32)
    w1cb = np.zeros((10, L * H), np.float32)
    w2p = np.zeros((H, L * H), np.float32)
    nw1 = np.zeros((H, L * 256), np.float32)
    nw2 = np.zeros((H, L * H), np.float32)
    for l in range(L):
        w1ab[:, 256 * l:256 * l + 128] = edge_w1[l][:128, :]
        w1ab[:, 256 * l + 128:256 * l + 256] = edge_w1[l][128:256, :]
        w1dz[:60, H * l:H * (l + 1)] = -edge_w1[l][265:325, :]
        w1cb[:9, H * l:H * (l + 1)] = edge_w1[l][256:265, :]
        w1cb[9, H * l:H * (l + 1)] = edge_b1[l]
        w2p[:, H * l:H * (l + 1)] = edge_w2[l]
        nw1[:, 256 * l:256 * l + 128] = node_w1[l][:128, :]
        nw1[:, 256 * l + 128:256 * l + 256] = node_w1[l][128:, :] / 32.0
        nw2[:, H * l:H * (l + 1)] = node_w2[l]
    w["w1ab"] = w1ab
    w["w1dz"] = w1dz
    w["w1cb"] = w1cb
    w["w2p"] = w2p
    w["nw1"] = nw1
    w["nw2"] = nw2
    w["b2t"] = np.ascontiguousarray(edge_b2.T)    # [128, 4]
    w["nb1t"] = np.ascontiguousarray(node_b1.T)   # [128, 4]
    w["nb2t"] = np.ascontiguousarray(node_b2.T)   # [128, 4]
    w["nemb"] = np.ascontiguousarray(node_emb)    # [100, 128]
    w["outw"] = np.ascontiguousarray(out_w / 32.0)
    return w


def _per_core_inputs(core, atom_types, frac_coords, lattices):
    d = {}
    ns = slice(NPC * core, NPC * (core + 1))
    gs = slice(BPC * core, BPC * (core + 1))
    d["fract"] = np.ascontiguousarray(frac_coords[ns].T)  # [3, 512]
    oh = np.zeros((100, NPC), np.float32)
    at = atom_types[ns].astype(np.int64) - 1
    oh[at, np.arange(NPC)] = 1.0
    d["onehott"] = oh
    A = lattices[gs]  # [16, 3, 3]
    lra = np.zeros((10, 3 * BPC), np.float32)
    lrb = np.zeros((10, 3 * BPC), np.float32)
    lra[:9] = np.broadcast_to(A.transpose(1, 0, 2)[:, None, :, :],
                              (3, 3, BPC, 3)).reshape(9, 3 * BPC)
    lrb[:9] = np.broadcast_to(A.transpose(1, 0, 2)[None, :, :, :],
                              (3, 3, BPC, 3)).reshape(9, 3 * BPC)
    # row 9 produces the constant-one row of vall after the j-reduce
    lra[9, 0::3] = 1.0
    lrb[9, 0::3] = 1.0
    d["lra"] = lra
    d["lrb"] = lrb
    return d


_SHAPES = dict(
    fract=(3, NPC), onehott=(100, NPC), lra=(10, 3 * BPC), lrb=(10, 3 * BPC),
    abselc=(64, EPG), zer32=(32, EPG), rf60=(3, 60), offv=(60, 1),
    w1ab=(H, L * 256), w1dz=(64, L * H), w1cb=(10, L * H), w2p=(H, L * H),
    nw1=(H, L * 256), nw2=(H, L * H),
    b2t=(H, L), nb1t=(H, L), nb2t=(H, L),
    nemb=(100, H), outw=(H, H),
)


# ----------------------------------------------------------------------------
# device kernel
# ----------------------------------------------------------------------------

def _emit(tc, nc, sbin, out_dram, ctx):
    import concourse.bass as bass
    from concourse import mybir

    f32 = mybir.dt.float32
    f32r = mybir.dt.float32r
    AF = mybir.ActivationFunctionType
    ALU = mybir.AluOpType
    AX = mybir.AxisListType

    # dtype for tensors consumed by the big matmuls: walrus requires fp32r
    # operands to be *produced* as fp32r, so the tiles carry the dtype.
    fbig = f32r if F32R_BIG else f32

    singles = ctx.enter_context(tc.tile_pool(name="singles", bufs=1))
    sigp = ctx.enter_context(tc.tile_pool(name="sigp", bufs=4))
    work = ctx.enter_context(tc.tile_pool(name="work", bufs=4))
    hpool = ctx.enter_context(tc.tile_pool(name="hpool", bufs=3))
    eps_pool = ctx.enter_context(tc.tile_pool(name="eps", bufs=3, space="PSUM"))
    sps_pool = ctx.enter_context(tc.tile_pool(name="sps", bufs=1, space="PSUM"))

    # ---- load all weights/constants into SBUF --------------------------------
    # emission order = rough DMA priority: the front of the kernel is gated on
    # phase-0/wave-0 dependencies, so those land first.
    _PRIO = ["fract", "rf60", "offv", "nemb", "onehott", "w1ab", "w1dz",
             "w1cb", "lra", "lrb"]
    _PRIO2 = ["w2p", "b2t", "nw1", "nw2", "nb1t", "nb2t", "outw"]
    sb = {}

    def load_sb(names):
        for name in names:
            dt = fbig if name in ("w1dz", "w2p") else f32
            t = singles.tile(list(_SHAPES[name]), dt, name=f"sb_{name}")
            nc.sync.dma_start(out=t, in_=sbin[name].ap())
            sb[name] = t

    load_sb(_PRIO)

    # disAB: per graph [128, 1024]; rows 0-59 sin-embedding (written later),
    # rows 60-63 zero, rows 64-127 the A/B one-hot selector.
    disab = singles.tile([128, BPC * EPG], fbig, name="disab")

    def disab_dma(g):
        nc.sync.dma_start(out=disab[64:128, EPG * g:EPG * (g + 1)],
                          in_=sbin["abselc"].ap())
        # rows 32-63 zeroed by DMA (60-63 stay zero; Sin overwrites 32-59;
        # 32-aligned partition starts are required, so zero all of 32-63)
        nc.sync.dma_start(out=disab[32:64, EPG * g:EPG * (g + 1)],
                          in_=sbin["zer32"].ap())

    for g in range(2):
        disab_dma(g)
    zero60 = singles.tile([60, 1], f32, name="zero60")
    nc.vector.memset(zero60, 0.0)
    # dummy no-op silu: makes walrus load `silu_and_others` (which also
    # contains sin) before the first Sin, avoiding a second table-set load
    dum60 = singles.tile([60, 1], f32, name="dum60")
    nc.scalar.activation(out=dum60, in_=zero60, func=AF.Silu, bias=zero60,
                         scale=1.0)

    # ---- phase 0: sinusoid embeddings (interleaved with the wave loop) -------
    # fracrT[r, n] = k_r * frac[n, d_r];  u = fracrT + off (16 / 16.25 cos);
    # t[r,(i,j)] = u[:,j] - fracrT[:,i] in (6, 27);
    # b = (t + 2^23) - 2^23 rounds t to the nearest integer (fp32 trick);
    # zneg = b - t in [-0.5, 0.5];  sin(2*pi*zneg) = -sin(2*pi*k*dx (+pi/2)),
    # and the leading minus is folded into the (negated) W1d weights.
    # Sin lives in the same ACT table set as Silu, so interleaving is free.
    RC = float(2 ** 23)
    p0state = {}

    def p0a(g):
        fr_ps = sps_pool.tile([60, NPG], f32, tag="ab", name="fr_ps")
        nc.tensor.matmul(fr_ps, lhsT=sb["rf60"],
                         rhs=sb["fract"][:, NPG * g:NPG * (g + 1)])
        fracrt = work.tile([60, NPG], f32, tag="fracrt", name="fracrt")
        nc.vector.tensor_copy(fracrt, fr_ps)
        uoff = work.tile([60, NPG], f32, tag="uoff", name="uoff")
        nc.vector.tensor_scalar_add(uoff, fracrt, sb["offv"])
        bcast_j = bass.AP(tensor=uoff.tensor, offset=uoff.offset,
                          ap=[uoff.ap[0], [0, NPG], [1, NPG]])
        bcast_i = bass.AP(tensor=fracrt.tensor, offset=fracrt.offset,
                          ap=[fracrt.ap[0], [1, NPG], [0, NPG]])
        tt = work.tile([60, NPG, NPG], f32, tag="tt", name="tt")
        nc.vector.tensor_sub(tt, bcast_j, bcast_i)
        p0state[g] = tt

    def p0b(g):
        tt = p0state.pop(g)
        tb = work.tile([60, EPG], f32, tag="tb", name="tb")
        nc.vector.tensor_scalar(tb.rearrange("p (i j) -> p i j", j=NPG), tt,
                                RC, RC, op0=ALU.add, op1=ALU.subtract)
        tz = work.tile([60, EPG], f32, tag="tz", name="tz")
        nc.gpsimd.tensor_sub(tz, tb,
                             tt.rearrange("p i j -> p (i j)"))
        nc.scalar.activation(out=disab[0:60, EPG * g:EPG * (g + 1)], in_=tz,
                             func=AF.Sin, bias=zero60,
                             scale=2.0 * float(np.pi))

    for g in range(4):
        p0a(g)
    for g in range(2, 8):
        disab_dma(g)
    load_sb(_PRIO2)
    for g in range(4):
        p0b(g)
        p0a(g + 4)
    for g in range(4, 8):
        p0b(g)
    # ABsel blocks for the second wave pair (deprioritized vs startup DMAs)
    for g in range(8, BPC):
        disab_dma(g)

    # ---- phase 0b: h init (embedding gather via one-hot matmul) --------------
    h4_ps = sps_pool.tile([H, NPC], f32, tag="node", name="h4_ps")
    nc.tensor.matmul(h4_ps, lhsT=sb["nemb"], rhs=sb["onehott"])
    hts = [[None] * (L + 1) for _ in range(WAVES)]
    for w in range(WAVES):
        ht0 = hpool.tile([H, 128], f32, tag=f"ht{w}", name=f"ht_{w}_0")
        nc.vector.tensor_copy(ht0, h4_ps[:, 128 * w:128 * (w + 1)])
        hts[w][0] = ht0

    # ---- phase 0c: lattice inner products -> per-(graph,layer) act biases ----
    vtmp = singles.tile([10, 3 * BPC], f32, name="vtmp")
    nc.vector.tensor_mul(vtmp, sb["lra"], sb["lrb"])
    vall = singles.tile([10, BPC], f32, name="vall")
    nc.vector.tensor_reduce(out=vall,
                            in_=vtmp.rearrange("p (b j) -> p b j", j=3),
                            axis=AX.X, op=ALU.add)
    biast = singles.tile([H, L * BPC], f32, name="biast")
    for l in range(L):
        b_ps = sps_pool.tile([H, BPC], f32, tag="ab", name="b_ps")
        nc.tensor.matmul(b_ps, lhsT=sb["w1cb"][:, H * l:H * (l + 1)], rhs=vall)
        nc.vector.tensor_copy(biast[:, BPC * l:BPC * (l + 1)], b_ps)

    # ---- phase 1: L rounds of message passing, interleaved wave pairs --------
    # Two waves advance in lockstep per layer so one wave's edge silus fill
    # the other wave's node-update join on the Activation engine.
    def wave_layer(w, l):
            ht = hts[w][l]
            agg = work.tile([H, 128], f32, tag="agg", name="agg")
            for g4 in range(4):
                g = 4 * w + g4
                # A/B node-feature matmuls into PSUM partitions 64..127
                ab_ps = sps_pool.tile([128, H], f32, tag="ab", name="ab_ps")
                nc.tensor.matmul(ab_ps[64:96, :],
                                 lhsT=ht[:, 32 * g4:32 * g4 + 32],
                                 rhs=sb["w1ab"][:, 256 * l:256 * l + 128],
                                 tile_position=(0, 64))
                nc.tensor.matmul(ab_ps[96:128, :],
                                 lhsT=ht[:, 32 * g4:32 * g4 + 32],
                                 rhs=sb["w1ab"][:, 256 * l + 128:256 * l + 256],
                                 tile_position=(0, 96))
                lhstp = work.tile([128, H], fbig, tag="lhstp", name="lhstp")
                nc.gpsimd.tensor_copy(lhstp[0:64, :],
                                      sb["w1dz"][:, H * l:H * (l + 1)])
                nc.vector.tensor_copy(lhstp[64:128, :], ab_ps[64:128, :])
                # pre-activation edge features [128, 1024]
                pre_ps = eps_pool.tile([H, EPG], f32, tag="pre", name="pre_ps")
                for cch in range(2):
                    cs = slice(512 * cch, 512 * (cch + 1))
                    nc.tensor.matmul(pre_ps[:, cs], lhsT=lhstp,
                                     rhs=disab[:, EPG * g + 512 * cch:
                                               EPG * g + 512 * (cch + 1)])
                sig1 = sigp.tile([H, EPG], fbig, tag="sig1", name="sig1")
                nc.scalar.activation(out=sig1, in_=pre_ps, func=AF.Silu,
                                     bias=biast[:, BPC * l + g:BPC * l + g + 1],
                                     scale=1.0)
                m2_ps = eps_pool.tile([H, EPG], f32, tag="pre", name="m2_ps")
                for cch in range(2):
                    cs = slice(512 * cch, 512 * (cch + 1))
                    nc.tensor.matmul(m2_ps[:, cs],
                                     lhsT=sb["w2p"][:, H * l:H * (l + 1)],
                                     rhs=sig1[:, cs])
                sig2 = sigp.tile([H, EPG], f32, tag="sig2", name="sig2")
                nc.scalar.activation(out=sig2, in_=m2_ps, func=AF.Silu,
                                     bias=sb["b2t"][:, l:l + 1], scale=1.0)
                nc.vector.tensor_reduce(
                    out=agg[:, 32 * g4:32 * g4 + 32],
                    in_=sig2.rearrange("p (i j) -> p i j", j=NPG),
                    axis=AX.X, op=ALU.add)
            # node update, 4 graphs at once
            u1_ps = sps_pool.tile([H, 128], f32, tag="node", name="u1_ps")
            nc.tensor.matmul(u1_ps, lhsT=sb["nw1"][:, 256 * l:256 * l + 128],
                             rhs=ht, start=True, stop=False)
            nc.tensor.matmul(u1_ps, lhsT=sb["nw1"][:, 256 * l + 128:256 * l + 256],
                             rhs=agg, start=False, stop=True)
            u1 = work.tile([H, 128], f32, tag="u1", name="u1")
            nc.scalar.activation(out=u1, in_=u1_ps, func=AF.Silu,
                                 bias=sb["nb1t"][:, l:l + 1], scale=1.0)
            u2_ps = sps_pool.tile([H, 128], f32, tag="node", name="u2_ps")
            nc.tensor.matmul(u2_ps, lhsT=sb["nw2"][:, H * l:H * (l + 1)], rhs=u1)
            u2 = work.tile([H, 128], f32, tag="u2", name="u2")
            nc.scalar.activation(out=u2, in_=u2_ps, func=AF.Silu,
                                 bias=sb["nb2t"][:, l:l + 1], scale=1.0)
            htn = hpool.tile([H, 128], f32, tag=f"ht{w}", name=f"ht_{w}_{l + 1}")
            nc.gpsimd.tensor_add(htn, ht, u2)
            hts[w][l + 1] = htn

    for wpair in range(WAVES // 2):
        wa, wb = 2 * wpair, 2 * wpair + 1
        for l in range(L):
            # prefetch the next pair's sinusoid embeddings while this runs
            if wpair == 0:
                if l == 0:
                    for gn in range(8, 12):
                        p0a(gn)
                elif l == 1:
                    for gn in range(8, 12):
                        p0b(gn)
                    for gn in range(12, 16):
                        p0a(gn)
                elif l == 2:
                    for gn in range(12, 16):
                        p0b(gn)
            wave_layer(wa, l)
            wave_layer(wb, l)

    # ---- phase 2: graph pooling + output projection --------------------------
    gt = singles.tile([H, BPC], f32, name="gt")
    for w in range(WAVES):
        nc.vector.tensor_reduce(
            out=gt[:, 4 * w:4 * (w + 1)],
            in_=hts[w][L].rearrange("p (b n) -> p b n", n=NPG),
            axis=AX.X, op=ALU.add)
    out_ps = sps_pool.tile([H, BPC], f32, tag="ab", name="out_ps")
    nc.tensor.matmul(out_ps, lhsT=sb["outw"], rhs=gt)
    outsb = singles.tile([H, BPC], f32, name="outsb")
    nc.vector.tensor_copy(outsb, out_ps)
    nc.sync.dma_start(out=out_dram.ap(), in_=outsb)


def _build():
    import concourse.bass as bass
    import concourse.bacc as bacc
    import concourse.tile as tile
    from concourse import mybir

    nc = bacc.Bacc("TRN2", target_bir_lowering=False, debug=False,
                   enable_asserts=False, num_devices=NCORES)
    fbig = mybir.dt.float32r if F32R_BIG else mybir.dt.float32
    sbin = {name: nc.dram_tensor(
                name, list(shape),
                fbig if name in ("w1dz", "w2p", "abselc", "zer32")
                else mybir.dt.float32,
                kind="ExternalInput")
            for name, shape in _SHAPES.items()}
    out_dram = nc.dram_tensor("outt", [H, BPC], mybir.dt.float32,
                              kind="ExternalOutput")
    with tile.TileContext(nc) as tc:
        with ExitStack() as ctx:
            _emit(tc, nc, sbin, out_dram, ctx)
    nc.compile()
    from concourse.bass_interp import get_hw_module
    nc.m = get_hw_module(nc.m)
    return nc


_NC = None


def _get_nc():
    global _NC
    if _NC is None:
        _NC = _build()
    return _NC


def _make_in_maps(inputs):
    atom_types = np.asarray(inputs["atom_types"]).astype(np.int32)
    frac_coords = np.asarray(inputs["frac_coords"]).astype(np.float32)
    lattices = np.asarray(inputs["lattices"]).astype(np.float32)
    shared = {}
    shared.update(_build_consts())
    shared.update(_pack_weights(
        np.asarray(inputs["edge_w1"], np.float32),
        np.asarray(inputs["edge_b1"], np.float32),
        np.asarray(inputs["edge_w2"], np.float32),
        np.asarray(inputs["edge_b2"], np.float32),
        np.asarray(inputs["node_w1"], np.float32),
        np.asarray(inputs["node_b1"], np.float32),
        np.asarray(inputs["node_w2"], np.float32),
        np.asarray(inputs["node_b2"], np.float32),
        np.asarray(inputs["node_emb"], np.float32),
        np.asarray(inputs["out_w"], np.float32)))
    in_maps = []
    for core in range(NCORES):
        m = dict(shared)
        m.update(_per_core_inputs(core, atom_types, frac_coords, lattices))
        for k in m:
            m[k] = np.ascontiguousarray(m[k], dtype=np.float32)
        in_maps.append(m)
    return in_maps


_EXEC = None


def _get_exec():
    """Build (once) a jitted PJRT callable running the NEFF on all 8 cores."""
    global _EXEC
    if _EXEC is not None:
        return _EXEC
    import jax
    from jax.sharding import Mesh, PartitionSpec
    from jax.experimental.shard_map import shard_map
    from concourse import bass2jax, mybir

    bass2jax.install_neuronx_cc_hook()
    nc = _get_nc()
    partition_name = (nc.partition_id_tensor.name
                      if nc.partition_id_tensor else None)
    in_names, out_names, out_avals = [], [], []
    for alloc in nc.m.functions[0].allocations:
        if not isinstance(alloc, mybir.MemoryLocationSet):
            continue
        name = alloc.memorylocations[0].name
        if alloc.kind == "ExternalInput":
            if name != partition_name:
                in_names.append(name)
        elif alloc.kind == "ExternalOutput":
            out_names.append(name)
            out_avals.append(jax.core.ShapedArray(
                tuple(alloc.tensor_shape), mybir.dt.np(alloc.dtype)))
    n_params = len(in_names)
    all_in_names = list(in_names) + list(out_names)
    if partition_name is not None:
        all_in_names.append(partition_name)

    def _body(*args):
        operands = list(args)
        if partition_name is not None:
            operands.append(bass2jax.partition_id_tensor())
        outs = bass2jax._bass_exec_p.bind(
            *operands,
            out_avals=tuple(out_avals),
            in_names=tuple(all_in_names),
            out_names=tuple(out_names),
            lowering_input_output_aliases=(),
            sim_require_finite=True,
            sim_require_nnan=True,
            nc=nc,
        )
        return tuple(outs)

    devices = jax.devices()[:NCORES]
    mesh = Mesh(np.asarray(devices), ("core",))
    n_outs = len(out_names)
    in_specs = (PartitionSpec("core"),) * (n_params + n_outs)
    out_specs = (PartitionSpec("core"),) * n_outs
    fn = jax.jit(shard_map(_body, mesh=mesh, in_specs=in_specs,
                           out_specs=out_specs, check_rep=False),
                 keep_unused=True)
    _EXEC = (fn, in_names, out_names, out_avals, mesh)
    return _EXEC


def _device_args(inputs):
    import jax
    from jax.sharding import NamedSharding, PartitionSpec
    fn, in_names, out_names, out_avals, mesh = _get_exec()
    in_maps = _make_in_maps(inputs)
    concat_in = [np.concatenate([in_maps[c][name] for c in range(NCORES)],
                                axis=0) for name in in_names]
    concat_zeros = [np.zeros((NCORES * a.shape[0], *a.shape[1:]), a.dtype)
                    for a in out_avals]
    sh = NamedSharding(mesh, PartitionSpec("core"))
    return [jax.device_put(a, sh) for a in concat_in + concat_zeros]


def _gather_out(out_arrs):
    outt = np.asarray(out_arrs[0]).reshape(NCORES, H, BPC)
    out = np.zeros((B, H), np.float32)
    for core in range(NCORES):
        out[BPC * core:BPC * (core + 1), :] = outt[core].T
    return out


def _run(inputs):
    import jax
    fn = _get_exec()[0]
    args = _device_args(inputs)
    out_arrs = fn(*args)
    jax.block_until_ready(out_arrs)
    return _gather_out(out_arrs), (fn, args)


def kernel(**inputs) -> np.ndarray:
    out, _ = _run(inputs)
    return out


# revision 48
# speedup vs baseline: 12.0329x; 11.0158x over previous
"""Trainium2 Bass kernel for CSPCPCPNet-style GNN message passing.

Graph structure (from the model): B=128 independent graphs, 32 nodes each,
fully-connected edges (incl. self-loops) that never cross graphs, nodes/edges
laid out contiguously per graph.  Edge e = g*1024 + i*32 + j has src=g*32+i,
dst=g*32+j.  The output only depends on the *set* of edges (aggregations are
permutation invariant), so the kernel uses this structure directly.

Sharding: 16 graphs per NeuronCore x 8 cores, MLP weights replicated,
no collectives.  Everything on-chip is kept transposed: features on the
128 partitions, edges/nodes along the free dimension.
"""

import os
import numpy as np
from contextlib import ExitStack

H = 128
L = 4
B = 128
NPG = 32
EPG = NPG * NPG  # 1024
NFREQ = 10
NCORES = 8
BPC = B // NCORES  # 16 graphs per core
NPC = BPC * NPG  # 512 nodes per core
WAVES = BPC // 4  # waves of 4 graphs

F32R_BIG = True  # use float32r (reduced precision, ~4x faster) for big matmuls


# ----------------------------------------------------------------------------
# host-side constant / weight packing (all arrays already in SBUF layout [P, F])
# ----------------------------------------------------------------------------

def _build_consts():
    c = {}
    # ABsel [64, 1024]: rows 0-31 select src i, rows 32-63 select dst j
    absel = np.zeros((64, EPG), np.float32)
    for i in range(NPG):
        absel[i, i * NPG:(i + 1) * NPG] = 1.0
        absel[32 + i, i::NPG] = 1.0
    c["abselc"] = absel
    # Rf60 [3, 60]: Rf60[d, d*10+k] = Rf60[d, 30+d*10+k] = k  (frequency / 2pi).
    # The kernel computes t = k*(x_j - x_i) + 16(+0.25 for cos) > 0, reduces
    # z = t mod 1, and evaluates sin(2*pi*z - pi) = -sin(2*pi*k*dx (+ pi/2));
    # the leading minus is folded into the (negated) W1d weights.
    rf = np.zeros((3, 60), np.float32)
    for d in range(3):
        for k in range(NFREQ):
            rf[d, d * NFREQ + k] = float(k)
            rf[d, 30 + d * NFREQ + k] = float(k)
    c["rf60"] = rf
    offv = np.full((60, 1), 16.0, np.float32)
    offv[30:] = 16.25
    c["offv"] = offv
    c["zer32"] = np.zeros((32, EPG), np.float32)
    return c


def _pack_weights(edge_w1, edge_b1, edge_w2, edge_b2,
                  node_w1, node_b1, node_w2, node_b2, node_emb, out_w):
    w = {}
    w1ab = np.zeros((H, L * 256), np.float32)
    w1dz = np.zeros((64, L * H), np.float32)
    w1cb = np.zeros((10, L * H), np.float32)
    w2p = np.zeros((H, L * H), np.float32)
    nw1 = np.zeros((H, L * 256), np.float32)
    nw2 = np.zeros((H, L * H), np.float32)
    for l in range(L):
        w1ab[:, 256 * l:256 * l + 128] = edge_w1[l][:128, :]
        w1ab[:, 256 * l + 128:256 * l + 256] = edge_w1[l][128:256, :]
        w1dz[:60, H * l:H * (l + 1)] = -edge_w1[l][265:325, :]
        w1cb[:9, H * l:H * (l + 1)] = edge_w1[l][256:265, :]
        w1cb[9, H * l:H * (l + 1)] = edge_b1[l]
        w2p[:, H * l:H * (l + 1)] = edge_w2[l]
        nw1[:, 256 * l:256 * l + 128] = node_w1[l][:128, :]
        nw1[:, 256 * l + 128:256 * l + 256] = node_w1[l][128:, :] / 32.0
        nw2[:, H * l:H * (l + 1)] = node_w2[l]
    w["w1ab"] = w1ab
    w["w1dz"] = w1dz
    w["w1cb"] = w1cb
    w["w2p"] = w2p
    w["nw1"] = nw1
    w["nw2"] = nw2
    w["b2t"] = np.ascontiguousarray(edge_b2.T)    # [128, 4]
    w["nb1t"] = np.ascontiguousarray(node_b1.T)   # [128, 4]
    w["nb2t"] = np.ascontiguousarray(node_b2.T)   # [128, 4]
    w["nemb"] = np.ascontiguousarray(node_emb)    # [100, 128]
    w["outw"] = np.ascontiguousarray(out_w / 32.0)
    return w


def _per_core_inputs(core, atom_types, frac_coords, lattices):
    d = {}
    ns = slice(NPC * core, NPC * (core + 1))
    gs = slice(BPC * core, BPC * (core + 1))
    d["fract"] = np.ascontiguousarray(frac_coords[ns].T)  # [3, 512]
    oh = np.zeros((100, NPC), np.float32)
    at = atom_types[ns].astype(np.int64) - 1
    oh[at, np.arange(NPC)] = 1.0
    d["onehott"] = oh
    A = lattices[gs]  # [16, 3, 3]
    lra = np.zeros((10, 3 * BPC), np.float32)
    lrb = np.zeros((10, 3 * BPC), np.float32)
    lra[:9] = np.broadcast_to(A.transpose(1, 0, 2)[:, None, :, :],
                              (3, 3, BPC, 3)).reshape(9, 3 * BPC)
    lrb[:9] = np.broadcast_to(A.transpose(1, 0, 2)[None, :, :, :],
                              (3, 3, BPC, 3)).reshape(9, 3 * BPC)
    # row 9 produces the constant-one row of vall after the j-reduce
    lra[9, 0::3] = 1.0
    lrb[9, 0::3] = 1.0
    d["lra"] = lra
    d["lrb"] = lrb
    return d


_SHAPES = dict(
    fract=(3, NPC), onehott=(100, NPC), lra=(10, 3 * BPC), lrb=(10, 3 * BPC),
    abselc=(64, EPG), zer32=(32, EPG), rf60=(3, 60), offv=(60, 1),
    w1ab=(H, L * 256), w1dz=(64, L * H), w1cb=(10, L * H), w2p=(H, L * H),
    nw1=(H, L * 256), nw2=(H, L * H),
    b2t=(H, L), nb1t=(H, L), nb2t=(H, L),
    nemb=(100, H), outw=(H, H),
)


# ----------------------------------------------------------------------------
# device kernel
# ----------------------------------------------------------------------------

def _emit(tc, nc, sbin, out_dram, ctx):
    import concourse.bass as bass
    from concourse import mybir

    f32 = mybir.dt.float32
    f32r = mybir.dt.float32r
    AF = mybir.ActivationFunctionType
    ALU = mybir.AluOpType
    AX = mybir.AxisListType

    # dtype for tensors consumed by the big matmuls: walrus requires fp32r
    # operands to be *produced* as fp32r, so the tiles carry the dtype.
    fbig = f32r if F32R_BIG else f32

    singles = ctx.enter_context(tc.tile_pool(name="singles", bufs=1))
    sigp = ctx.enter_context(tc.tile_pool(name="sigp", bufs=4))
    work = ctx.enter_context(tc.tile_pool(name="work", bufs=4))
    hpool = ctx.enter_context(tc.tile_pool(name="hpool", bufs=3))
    eps_pool = ctx.enter_context(tc.tile_pool(name="eps", bufs=3, space="PSUM"))
    sps_pool = ctx.enter_context(tc.tile_pool(name="sps", bufs=1, space="PSUM"))

    # ---- load all weights/constants into SBUF --------------------------------
    # emission order = rough DMA priority: the front of the kernel is gated on
    # phase-0/wave-0 dependencies, so those land first.
    _PRIO = ["fract", "rf60", "offv"]
    _PRIO1 = ["nemb", "onehott", "w1ab", "w1dz", "w1cb", "lra", "lrb"]
    _PRIO2 = ["w2p", "b2t", "nw1", "nw2", "nb1t", "nb2t", "outw"]
    sb = {}

    def load_sb(names):
        for name in names:
            dt = fbig if name in ("w1dz", "w2p") else f32
            t = singles.tile(list(_SHAPES[name]), dt, name=f"sb_{name}")
            nc.sync.dma_start(out=t, in_=sbin[name].ap())
            sb[name] = t

    load_sb(_PRIO)

    # disAB: per graph [128, 1024]; rows 0-59 sin-embedding (written later),
    # rows 60-63 zero, rows 64-127 the A/B one-hot selector.
    disab = singles.tile([128, BPC * EPG], fbig, name="disab")

    def disab_dma(g):
        nc.sync.dma_start(out=disab[64:128, EPG * g:EPG * (g + 1)],
                          in_=sbin["abselc"].ap())
        # rows 32-63 zeroed by DMA (60-63 stay zero; Sin overwrites 32-59;
        # 32-aligned partition starts are required, so zero all of 32-63)
        nc.sync.dma_start(out=disab[32:64, EPG * g:EPG * (g + 1)],
                          in_=sbin["zer32"].ap())

    # graph 0/1's sin-path DMAs come before the bulk weight loads: the first
    # Sin ops gate the whole Activation pipeline ramp
    for g in range(2):
        disab_dma(g)
    load_sb(_PRIO1)
    zero60 = singles.tile([60, 1], f32, name="zero60")
    nc.vector.memset(zero60, 0.0)
    # dummy no-op silu: makes walrus load `silu_and_others` (which also
    # contains sin) before the first Sin, avoiding a second table-set load
    dum60 = singles.tile([60, 1], f32, name="dum60")
    nc.scalar.activation(out=dum60, in_=zero60, func=AF.Silu, bias=zero60,
                         scale=1.0)

    # ---- phase 0: sinusoid embeddings (interleaved with the wave loop) -------
    # fracrT[r, n] = k_r * frac[n, d_r];  u = fracrT + off (16 / 16.25 cos);
    # t[r,(i,j)] = u[:,j] - fracrT[:,i] in (6, 27);
    # b = (t + 2^23) - 2^23 rounds t to the nearest integer (fp32 trick);
    # zneg = b - t in [-0.5, 0.5];  sin(2*pi*zneg) = -sin(2*pi*k*dx (+pi/2)),
    # and the leading minus is folded into the (negated) W1d weights.
    # Sin lives in the same ACT table set as Silu, so interleaving is free.
    RC = float(2 ** 23)
    p0state = {}

    def p0a(g):
        fr_ps = sps_pool.tile([60, NPG], f32, tag="ab", name="fr_ps")
        nc.tensor.matmul(fr_ps, lhsT=sb["rf60"],
                         rhs=sb["fract"][:, NPG * g:NPG * (g + 1)])
        fracrt = work.tile([60, NPG], f32, tag="fracrt", name="fracrt")
        nc.vector.tensor_copy(fracrt, fr_ps)
        uoff = work.tile([60, NPG], f32, tag="uoff", name="uoff")
        nc.vector.tensor_scalar_add(uoff, fracrt, sb["offv"])
        bcast_j = bass.AP(tensor=uoff.tensor, offset=uoff.offset,
                          ap=[uoff.ap[0], [0, NPG], [1, NPG]])
        bcast_i = bass.AP(tensor=fracrt.tensor, offset=fracrt.offset,
                          ap=[fracrt.ap[0], [1, NPG], [0, NPG]])
        tt = work.tile([60, NPG, NPG], f32, tag="tt", name="tt")
        nc.vector.tensor_sub(tt, bcast_j, bcast_i)
        p0state[g] = tt

    def p0b(g):
        tt = p0state.pop(g)
        tb = work.tile([60, EPG], f32, tag="tb", name="tb")
        nc.vector.tensor_scalar(tb.rearrange("p (i j) -> p i j", j=NPG), tt,
                                RC, RC, op0=ALU.add, op1=ALU.subtract)
        tz = work.tile([60, EPG], f32, tag="tz", name="tz")
        nc.gpsimd.tensor_sub(tz, tb,
                             tt.rearrange("p i j -> p (i j)"))
        nc.scalar.activation(out=disab[0:60, EPG * g:EPG * (g + 1)], in_=tz,
                             func=AF.Sin, bias=zero60,
                             scale=2.0 * float(np.pi))

    for g in range(4):
        p0a(g)
    for g in range(2, 8):
        disab_dma(g)
    load_sb(_PRIO2)
    for g in range(4):
        p0b(g)
        p0a(g + 4)
    for g in range(4, 8):
        p0b(g)
    # ABsel blocks for the second wave pair (deprioritized vs startup DMAs)
    for g in range(8, BPC):
        disab_dma(g)

    # ---- phase 0b: h init (embedding gather via one-hot matmul) --------------
    h4_ps = sps_pool.tile([H, NPC], f32, tag="node", name="h4_ps")
    nc.tensor.matmul(h4_ps, lhsT=sb["nemb"], rhs=sb["onehott"])
    hts = [[None] * (L + 1) for _ in range(WAVES)]
    for w in range(WAVES):
        ht0 = hpool.tile([H, 128], f32, tag=f"ht{w}", name=f"ht_{w}_0")
        nc.vector.tensor_copy(ht0, h4_ps[:, 128 * w:128 * (w + 1)])
        hts[w][0] = ht0

    # ---- phase 0c: lattice inner products -> per-(graph,layer) act biases ----
    vtmp = singles.tile([10, 3 * BPC], f32, name="vtmp")
    nc.vector.tensor_mul(vtmp, sb["lra"], sb["lrb"])
    vall = singles.tile([10, BPC], f32, name="vall")
    nc.vector.tensor_reduce(out=vall,
                            in_=vtmp.rearrange("p (b j) -> p b j", j=3),
                            axis=AX.X, op=ALU.add)
    biast = singles.tile([H, L * BPC], f32, name="biast")
    for l in range(L):
        b_ps = sps_pool.tile([H, BPC], f32, tag="ab", name="b_ps")
        nc.tensor.matmul(b_ps, lhsT=sb["w1cb"][:, H * l:H * (l + 1)], rhs=vall)
        nc.vector.tensor_copy(biast[:, BPC * l:BPC * (l + 1)], b_ps)

    # ---- phase 1: L rounds of message passing, interleaved wave pairs --------
    # Two waves advance in lockstep per layer so one wave's edge silus fill
    # the other wave's node-update join on the Activation engine.
    def wave_layer(w, l):
            ht = hts[w][l]
            agg = work.tile([H, 128], f32, tag="agg", name="agg")
            for g4 in range(4):
                g = 4 * w + g4
                # A/B node-feature matmuls into PSUM partitions 64..127
                ab_ps = sps_pool.tile([128, H], f32, tag="ab", name="ab_ps")
                nc.tensor.matmul(ab_ps[64:96, :],
                                 lhsT=ht[:, 32 * g4:32 * g4 + 32],
                                 rhs=sb["w1ab"][:, 256 * l:256 * l + 128],
                                 tile_position=(0, 64))
                nc.tensor.matmul(ab_ps[96:128, :],
                                 lhsT=ht[:, 32 * g4:32 * g4 + 32],
                                 rhs=sb["w1ab"][:, 256 * l + 128:256 * l + 256],
                                 tile_position=(0, 96))
                lhstp = work.tile([128, H], fbig, tag="lhstp", name="lhstp")
                nc.gpsimd.tensor_copy(lhstp[0:64, :],
                                      sb["w1dz"][:, H * l:H * (l + 1)])
                nc.vector.tensor_copy(lhstp[64:128, :], ab_ps[64:128, :])
                # pre-activation edge features [128, 1024]
                pre_ps = eps_pool.tile([H, EPG], f32, tag="pre", name="pre_ps")
                for cch in range(2):
                    cs = slice(512 * cch, 512 * (cch + 1))
                    nc.tensor.matmul(pre_ps[:, cs], lhsT=lhstp,
                                     rhs=disab[:, EPG * g + 512 * cch:
                                               EPG * g + 512 * (cch + 1)])
                sig1 = sigp.tile([H, EPG], fbig, tag="sig1", name="sig1")
                nc.scalar.activation(out=sig1, in_=pre_ps, func=AF.Silu,
                                     bias=biast[:, BPC * l + g:BPC * l + g + 1],
                                     scale=1.0)
                m2_ps = eps_pool.tile([H, EPG], f32, tag="pre", name="m2_ps")
                for cch in range(2):
                    cs = slice(512 * cch, 512 * (cch + 1))
                    nc.tensor.matmul(m2_ps[:, cs],
                                     lhsT=sb["w2p"][:, H * l:H * (l + 1)],
                                     rhs=sig1[:, cs])
                sig2 = sigp.tile([H, EPG], f32, tag="sig2", name="sig2")
                nc.scalar.activation(out=sig2, in_=m2_ps, func=AF.Silu,
                                     bias=sb["b2t"][:, l:l + 1], scale=1.0)
                nc.vector.tensor_reduce(
                    out=agg[:, 32 * g4:32 * g4 + 32],
                    in_=sig2.rearrange("p (i j) -> p i j", j=NPG),
                    axis=AX.X, op=ALU.add)
            # node update, 4 graphs at once
            u1_ps = sps_pool.tile([H, 128], f32, tag="node", name="u1_ps")
            nc.tensor.matmul(u1_ps, lhsT=sb["nw1"][:, 256 * l:256 * l + 128],
                             rhs=ht, start=True, stop=False)
            nc.tensor.matmul(u1_ps, lhsT=sb["nw1"][:, 256 * l + 128:256 * l + 256],
                             rhs=agg, start=False, stop=True)
            u1 = work.tile([H, 128], f32, tag="u1", name="u1")
            nc.scalar.activation(out=u1, in_=u1_ps, func=AF.Silu,
                                 bias=sb["nb1t"][:, l:l + 1], scale=1.0)
            u2_ps = sps_pool.tile([H, 128], f32, tag="node", name="u2_ps")
            nc.tensor.matmul(u2_ps, lhsT=sb["nw2"][:, H * l:H * (l + 1)], rhs=u1)
            u2 = work.tile([H, 128], f32, tag="u2", name="u2")
            nc.scalar.activation(out=u2, in_=u2_ps, func=AF.Silu,
                                 bias=sb["nb2t"][:, l:l + 1], scale=1.0)
            htn = hpool.tile([H, 128], f32, tag=f"ht{w}", name=f"ht_{w}_{l + 1}")
            nc.gpsimd.tensor_add(htn, ht, u2)
            hts[w][l + 1] = htn

    gt = singles.tile([H, BPC], f32, name="gt")
    for wpair in range(WAVES // 2):
        wa, wb = 2 * wpair, 2 * wpair + 1
        for l in range(L):
            # prefetch the next pair's sinusoid embeddings while this runs
            if wpair == 0:
                if l == 0:
                    for gn in range(8, 12):
                        p0a(gn)
                elif l == 1:
                    for gn in range(8, 12):
                        p0b(gn)
                    for gn in range(12, 16):
                        p0a(gn)
                elif l == 2:
                    for gn in range(12, 16):
                        p0b(gn)
            wave_layer(wa, l)
            wave_layer(wb, l)
            if l == L - 1:
                # pool each wave as soon as its final h is ready (tail overlap)
                for w in (wa, wb):
                    nc.vector.tensor_reduce(
                        out=gt[:, 4 * w:4 * (w + 1)],
                        in_=hts[w][L].rearrange("p (b n) -> p b n", n=NPG),
                        axis=AX.X, op=ALU.add)

    # ---- phase 2: output projection ------------------------------------------
    out_ps = sps_pool.tile([H, BPC], f32, tag="ab", name="out_ps")
    nc.tensor.matmul(out_ps, lhsT=sb["outw"], rhs=gt)
    outsb = singles.tile([H, BPC], f32, name="outsb")
    nc.vector.tensor_copy(outsb, out_ps)
    nc.sync.dma_start(out=out_dram.ap(), in_=outsb)


def _build():
    import concourse.bass as bass
    import concourse.bacc as bacc
    import concourse.tile as tile
    from concourse import mybir

    nc = bacc.Bacc("TRN2", target_bir_lowering=False, debug=False,
                   enable_asserts=False, num_devices=NCORES)
    fbig = mybir.dt.float32r if F32R_BIG else mybir.dt.float32
    sbin = {name: nc.dram_tensor(
                name, list(shape),
                fbig if name in ("w1dz", "w2p", "abselc", "zer32")
                else mybir.dt.float32,
                kind="ExternalInput")
            for name, shape in _SHAPES.items()}
    out_dram = nc.dram_tensor("outt", [H, BPC], mybir.dt.float32,
                              kind="ExternalOutput")
    with tile.TileContext(nc) as tc:
        with ExitStack() as ctx:
            _emit(tc, nc, sbin, out_dram, ctx)
    nc.compile()
    from concourse.bass_interp import get_hw_module
    nc.m = get_hw_module(nc.m)
    return nc


_NC = None


def _get_nc():
    global _NC
    if _NC is None:
        _NC = _build()
    return _NC


def _make_in_maps(inputs):
    atom_types = np.asarray(inputs["atom_types"]).astype(np.int32)
    frac_coords = np.asarray(inputs["frac_coords"]).astype(np.float32)
    lattices = np.asarray(inputs["lattices"]).astype(np.float32)
    shared = {}
    shared.update(_build_consts())
    shared.update(_pack_weights(
        np.asarray(inputs["edge_w1"], np.float32),
        np.asarray(inputs["edge_b1"], np.float32),
        np.asarray(inputs["edge_w2"], np.float32),
        np.asarray(inputs["edge_b2"], np.float32),
        np.asarray(inputs["node_w1"], np.float32),
        np.asarray(inputs["node_b1"], np.float32),
        np.asarray(inputs["node_w2"], np.float32),
        np.asarray(inputs["node_b2"], np.float32),
        np.asarray(inputs["node_emb"], np.float32),
        np.asarray(inputs["out_w"], np.float32)))
    in_maps = []
    for core in range(NCORES):
        m = dict(shared)
        m.update(_per_core_inputs(core, atom_types, frac_coords, lattices))
        for k in m:
            m[k] = np.ascontiguousarray(m[k], dtype=np.float32)
        in_maps.append(m)
    return in_maps


_EXEC = None


def _get_exec():
    """Build (once) a jitted PJRT callable running the NEFF on all 8 cores."""
    global _EXEC
    if _EXEC is not None:
        return _EXEC
    import jax
    from jax.sharding import Mesh, PartitionSpec
    from jax.experimental.shard_map import shard_map
    from concourse import bass2jax, mybir

    bass2jax.install_neuronx_cc_hook()
    nc = _get_nc()
    partition_name = (nc.partition_id_tensor.name
                      if nc.partition_id_tensor else None)
    in_names, out_names, out_avals = [], [], []
    for alloc in nc.m.functions[0].allocations:
        if not isinstance(alloc, mybir.MemoryLocationSet):
            continue
        name = alloc.memorylocations[0].name
        if alloc.kind == "ExternalInput":
            if name != partition_name:
                in_names.append(name)
        elif alloc.kind == "ExternalOutput":
            out_names.append(name)
            out_avals.append(jax.core.ShapedArray(
                tuple(alloc.tensor_shape), mybir.dt.np(alloc.dtype)))
    n_params = len(in_names)
    all_in_names = list(in_names) + list(out_names)
    if partition_name is not None:
        all_in_names.append(partition_name)

    def _body(*args):
        operands = list(args)
        if partition_name is not None:
            operands.append(bass2jax.partition_id_tensor())
        outs = bass2jax._bass_exec_p.bind(
            *operands,
            out_avals=tuple(out_avals),
            in_names=tuple(all_in_names),
            out_names=tuple(out_names),
            lowering_input_output_aliases=(),
            sim_require_finite=True,
            sim_require_nnan=True,
            nc=nc,
        )
        return tuple(outs)

    devices = jax.devices()[:NCORES]
    mesh = Mesh(np.asarray(devices), ("core",))
    n_outs = len(out_names)
    in_specs = (PartitionSpec("core"),) * (n_params + n_outs)
    out_specs = (PartitionSpec("core"),) * n_outs
    fn = jax.jit(shard_map(_body, mesh=mesh, in_specs=in_specs,
                           out_specs=out_specs, check_rep=False),
                 keep_unused=True)
    _EXEC = (fn, in_names, out_names, out_avals, mesh)
    return _EXEC


def _device_args(inputs):
    import jax
    from jax.sharding import NamedSharding, PartitionSpec
    fn, in_names, out_names, out_avals, mesh = _get_exec()
    in_maps = _make_in_maps(inputs)
    concat_in = [np.concatenate([in_maps[c][name] for c in range(NCORES)],
                                axis=0) for name in in_names]
    concat_zeros = [np.zeros((NCORES * a.shape[0], *a.shape[1:]), a.dtype)
                    for a in out_avals]
    sh = NamedSharding(mesh, PartitionSpec("core"))
    return [jax.device_put(a, sh) for a in concat_in + concat_zeros]


def _gather_out(out_arrs):
    outt = np.asarray(out_arrs[0]).reshape(NCORES, H, BPC)
    out = np.zeros((B, H), np.float32)
    for core in range(NCORES):
        out[BPC * core:BPC * (core + 1), :] = outt[core].T
    return out


def _run(inputs):
    import jax
    fn = _get_exec()[0]
    args = _device_args(inputs)
    out_arrs = fn(*args)
    jax.block_until_ready(out_arrs)
    return _gather_out(out_arrs), (fn, args)


def kernel(**inputs) -> np.ndarray:
    out, _ = _run(inputs)
    return out
